# revision 21
# baseline (speedup 1.0000x reference)
"""Trainium2 Bass kernel for a 3-layer GIN encoder (gnn_message_passing).

Reference computation (per layer l):
    agg_i = sum_{j -> i} z_j          (scatter-add over edges)
    h     = z + agg                   (GIN eps=0, folded in as self-edges)
    z     = relu(relu(h @ w1 + b1) @ w2 + b2)

Distribution strategy (8 NeuronCores, SPMD single program):
  * Nodes padded to N* = 10240 (80 chunks of 128); core c owns slots
    [1280c, 1280c+1280) (core 7: 1040 real rows + pad).
  * Edges partitioned by destination core; aggregation is local and runs on
    TensorE as a dense matmul with the transposed local adjacency-count
    matrix A' (includes +I self edges; max count 3 => exact in fp8e4).
  * fp8 everywhere in the aggregation: A' lives RESIDENT in SBUF as fp8e4
    ([128, 80, 1280] = 100KB/partition, loaded once) and activations z are
    stored/exchanged as fp8e4.  Aggregation matmuls use DoubleRow perf mode
    (contraction 256 per instruction, ~2x bf16 throughput); e4m3
    quantization of z adds ~0.6% final rel err (tolerance 2e-2).
  * MLP in plain bf16 (h from PSUM cast to bf16).  MLP2 is computed as
    s1.T @ w2 so z comes out ROW-major -- no TensorE transposes anywhere.
    The per-feature bias b2 is applied with a DVE add against a
    host-broadcast [128, 256] tile, relu+fp8-cast on ACT.
  * Layer boundary halo exchange: z8 tiles stream to DRAM and ship via 5
    fine-grained AllGathers per layer (one per pair-group j), so the next
    layer's aggregation (ordered j-major) starts as soon as AG j=0 lands,
    overlapping collectives with compute.
"""

import os
import sys

sys.path.insert(0, "/opt/trn_rl_repo")

import numpy as np
import ml_dtypes

BF16 = ml_dtypes.bfloat16
E4M3 = ml_dtypes.float8_e4m3  # TRN float8e4 (max +-240)

P = 128
NCORES = 8
NSTAR = 10240           # padded node count (80 chunks)
KC = NSTAR // P         # 80 chunks
NPAIR = KC // 2         # 40 DoubleRow pair-chunks
NPC = NSTAR // NCORES   # 1280 slots per core
MT = NPC // P           # 10 row tiles per core
NJ = MT // 2            # 5 pair-groups (AllGathers) per layer
NG = [(0, 512), (512, 512), (1024, 256)]   # agg free-dim groups (PSUM banks)
SG = [(0, 512), (512, 512), (1024, 256)]   # MLP slot groups

_BUILD_CACHE: dict = {}


# --------------------------------------------------------------------------
# host-side preprocessing
# --------------------------------------------------------------------------

def _config(inputs):
    x = inputs["x"]
    N, DIN = int(x.shape[0]), int(x.shape[1])
    L = 0
    while f"w1_{L}" in inputs:
        L += 1
    DH = int(inputs["w1_0"].shape[1])
    return dict(N=N, DIN=DIN, DH=DH, L=L)


def _prep_adj(edge_index, N):
    """Per-core transposed local adjacency, fp8e4, [128, KC, NPC]:
    a8[c][p, g, s] = #edges (src = g*128+p) -> (dst = c*NPC + s), + self."""
    src = np.asarray(edge_index[0], dtype=np.int64)
    dst = np.asarray(edge_index[1], dtype=np.int64)
    self_ix = np.arange(N, dtype=np.int64)
    allsrc = np.concatenate([src, self_ix])
    alldst = np.concatenate([dst, self_ix])

    at = np.zeros((NSTAR, NSTAR), np.uint8)
    np.add.at(at, (allsrc, alldst), 1)

    out = []
    for c in range(NCORES):
        sl = at[:, c * NPC:(c + 1) * NPC]                      # [NSTAR, NPC]
        sl = sl.reshape(KC, P, NPC).transpose(1, 0, 2)         # [P, KC, NPC]
        out.append(np.ascontiguousarray(sl).astype(E4M3))
    return out


def _prep_x8(x, N, DIN):
    xp = np.zeros((NSTAR, DIN), np.float32)
    xp[:N] = np.asarray(x, dtype=np.float32)
    x8 = xp.astype(E4M3).reshape(KC, P, DIN).transpose(1, 0, 2)
    return np.ascontiguousarray(x8)                            # [P, KC, DIN]


# --------------------------------------------------------------------------
# bass program
# --------------------------------------------------------------------------

def _build(DIN, DH, L):
    from concourse import bacc, mybir, tile

    f32 = mybir.dt.float32
    bf = mybir.dt.bfloat16
    f8 = mybir.dt.float8e4
    ADD = mybir.AluOpType.add
    RELU = mybir.ActivationFunctionType.Relu
    DR = mybir.MatmulPerfMode.DoubleRow

    NKT2 = DH // P  # hidden-dim 128-blocks (2)

    nc = bacc.Bacc(num_devices=NCORES)

    a8in = nc.dram_tensor("a8", [P, KC, NPC], f8, kind="ExternalInput")
    x8in = nc.dram_tensor("x8", [P, KC, DIN], f8, kind="ExternalInput")
    ccwin = nc.dram_tensor("ccw", [P, 64], f8, kind="ExternalInput")
    win = {}
    for l in range(L):
        din = DIN if l == 0 else DH
        win[("w1", l)] = nc.dram_tensor(f"w1b_{l}", [din, DH], bf, kind="ExternalInput")
        win[("w2", l)] = nc.dram_tensor(f"w2b_{l}", [DH, DH], bf, kind="ExternalInput")
        win[("b1", l)] = nc.dram_tensor(f"b1_{l}", [DH, 1], f32, kind="ExternalInput")
        win[("b2", l)] = nc.dram_tensor(f"b2b_{l}", [P, DH], f32, kind="ExternalInput")
    zout = nc.dram_tensor("zout", [NPC, DH], f32, kind="ExternalOutput")

    with tile.TileContext(nc) as tc:
        with tc.tile_pool(name="const", bufs=1) as cp, \
             tc.tile_pool(name="hpool", bufs=1) as hp, \
             tc.tile_pool(name="spool", bufs=2) as sp, \
             tc.tile_pool(name="tpool", bufs=2) as tp, \
             tc.tile_pool(name="z8pool", bufs=3) as z8p, \
             tc.tile_pool(name="hpsum", bufs=1, space="PSUM") as hpsum, \
             tc.tile_pool(name="mlppsum", bufs=2, space="PSUM") as mlppool, \
             tc.tile_pool(name="drampool", bufs=1, space="DRAM") as dp:

            # ---------------- resident tensors ----------------
            # activation tables (fp8, chunk-major); x8 loads FIRST so the
            # layer-0 aggregation can start as soon as adjacency pairs land
            zsb0 = cp.tile([P, KC, DIN], f8, name="zsb0")
            nc.scalar.dma_start(out=zsb0[:], in_=x8in[:, :, :])
            zsb8 = cp.tile([P, KC, DH], f8, name="zsb8")

            # warm up the CC stream so the first real AllGather doesn't pay
            # the cold-start cost (runs concurrent with the adjacency load)
            ccsrc = dp.tile([P, 64], f8, name="ccsrc")
            nc.sync.dma_start(out=ccsrc[:, :], in_=ccwin[:, :])
            ccwarm = [dp.tile([P * NCORES, 64], f8, name=f"ccwarm{i}",
                              addr_space="Shared") for i in range(2)]
            with tc.high_priority():
                for i in range(2):
                    nc.gpsimd.collective_compute(
                        "AllGather", mybir.AluOpType.bypass,
                        replica_groups=[list(range(NCORES))],
                        ins=[ccsrc[:, :].opt()],
                        outs=[ccwarm[i][:, :].opt()],
                    )

            wt = {}
            for l in range(L):
                din = DIN if l == 0 else DH
                nkt = din // P
                t = cp.tile([P, nkt * DH], bf, name=f"w1t{l}")
                for kt in range(nkt):
                    nc.gpsimd.dma_start(
                        out=t[:, kt * DH:(kt + 1) * DH],
                        in_=win[("w1", l)][kt * P:(kt + 1) * P, :])
                wt[("w1", l)] = t
                t = cp.tile([P, NKT2 * DH], bf, name=f"w2t{l}")
                for kt in range(NKT2):
                    nc.gpsimd.dma_start(
                        out=t[:, kt * DH:(kt + 1) * DH],
                        in_=win[("w2", l)][kt * P:(kt + 1) * P, :])
                wt[("w2", l)] = t
                t = cp.tile([P, NKT2], f32, name=f"b1t{l}")
                for kb in range(NKT2):
                    nc.gpsimd.dma_start(
                        out=t[:, kb:kb + 1],
                        in_=win[("b1", l)][kb * P:(kb + 1) * P, :])
                wt[("b1", l)] = t
                t = cp.tile([P, DH], f32, name=f"b2t{l}")
                nc.gpsimd.dma_start(out=t[:], in_=win[("b2", l)][:, :])
                wt[("b2", l)] = t

            # adjacency: resident fp8, streamed in once in 1.3MB batches;
            # mostly on the two HWDGE queues (layer-0 agg consumes pairs in
            # order as they arrive), last two batches on gpsimd SWDGE
            acache = cp.tile([P, KC, NPC], f8, name="acache")
            qmap = [nc.scalar, nc.sync] * 4 + [nc.gpsimd, nc.gpsimd]
            for bt in range(NPAIR // 4):
                qmap[bt].dma_start(
                    out=acache[:, 8 * bt:8 * bt + 8, :],
                    in_=a8in[:, 8 * bt:8 * bt + 8, :])

            # layer-boundary exchange buffers; AGS = tile ranges per
            # AllGather: a small first AG to unblock the next layer's agg
            # early, then two larger ones (better collective bandwidth).
            # zloc8 groups are partition-major [P, tiles, DH] so the
            # gathered blocks fill zsb8 with large contiguous DMA lines.
            AGS = [(0, 2), (2, 6), (6, 10)]
            zloc8 = [[dp.tile([P, b - a, DH], f8, name=f"zloc8_{l}_{gi}")
                      for gi, (a, b) in enumerate(AGS)]
                     for l in range(L - 1)]
            zfull8 = [[dp.tile([NCORES * P, (b - a) * DH], f8,
                               name=f"zfull8_{l}_{gi}", addr_space="Shared")
                       for gi, (a, b) in enumerate(AGS)]
                      for l in range(L - 1)]

            # ---------------- layers ----------------
            for l in range(L):
                din = DIN if l == 0 else DH
                nkt = din // P
                last = (l == L - 1)
                zsb = zsb0 if l == 0 else zsb8

                # --- aggregation: h.T = z8.T @ A'  (DoubleRow fp8)
                hps = [hpsum.tile([P, len(NG) * 512], f32,
                                  name=f"hps{mf}_{l}", tag=f"hps{mf}")
                       for mf in range(nkt)]
                if l == 0:
                    order = list(range(NPAIR))           # a8 load order
                else:
                    # AG j covers pairs {5c+j}: consume in arrival order
                    order = [NJ * c + j for j in range(NJ) for c in range(NCORES)]
                for cnt, kk in enumerate(order):
                    for mf in range(nkt):
                        for gi, (n0, nn) in enumerate(NG):
                            nc.tensor.matmul(
                                out=hps[mf][:, gi * 512: gi * 512 + nn],
                                lhsT=zsb[:, 2 * kk:2 * kk + 2,
                                         mf * P:(mf + 1) * P],
                                rhs=acache[:, 2 * kk:2 * kk + 2, n0:n0 + nn],
                                start=(cnt == 0), stop=(cnt == NPAIR - 1),
                                perf_mode=DR,
                            )

                # --- h -> bf16 (feature-major) for the MLP
                hhi = [hp.tile([P, NPC], bf, name=f"hhi{mf}_{l}", tag=f"hhi{mf}")
                       for mf in range(nkt)]
                for mf in range(nkt):
                    for gi, (n0, nn) in enumerate(NG):
                        nc.vector.tensor_copy(
                            out=hhi[mf][:, n0:n0 + nn],
                            in_=hps[mf][:, gi * 512: gi * 512 + nn])

                # --- MLP over slot groups; MLP2 emits row-major z tiles
                for (g0, rows) in SG:
                    s1 = []
                    for kb in range(NKT2):
                        p1 = mlppool.tile([P, 512], f32,
                                          name=f"p1_{l}_{g0}_{kb}", tag="mlp")
                        for mf in range(nkt):
                            nc.tensor.matmul(
                                out=p1[:, :rows],
                                lhsT=wt[("w1", l)][:, mf * DH + kb * P:
                                                   mf * DH + (kb + 1) * P],
                                rhs=hhi[mf][:, g0:g0 + rows],
                                start=(mf == 0), stop=(mf == nkt - 1))
                        st = sp.tile([P, 512], bf, name=f"s1_{l}_{g0}_{kb}",
                                     tag=f"s1{kb}")
                        nc.scalar.activation(
                            out=st[:, :rows], in_=p1[:, :rows], func=RELU,
                            bias=wt[("b1", l)][:, kb:kb + 1])
                        s1.append(st)

                    for m in range(g0 // P, (g0 + rows) // P):
                        off = m * P - g0
                        p2 = mlppool.tile([P, DH], f32,
                                          name=f"p2_{l}_{m}", tag="mlp")
                        for kb in range(NKT2):
                            nc.tensor.matmul(
                                out=p2[:],
                                lhsT=s1[kb][:, off:off + P],
                                rhs=wt[("w2", l)][:, kb * DH:(kb + 1) * DH],
                                start=(kb == 0), stop=(kb == NKT2 - 1))
                        # z = relu(p2 + b2)  (b2 varies along free dim)
                        tsum = tp.tile([P, DH], f32, name=f"ts_{l}_{m}", tag="ts")
                        nc.vector.tensor_tensor(
                            out=tsum[:], in0=p2[:], in1=wt[("b2", l)][:], op=ADD)
                        if last:
                            zf = z8p.tile([P, DH], f32, name=f"zf_{m}", tag="zf")
                            nc.scalar.activation(out=zf[:], in_=tsum[:], func=RELU)
                            nc.sync.dma_start(
                                out=zout[m * P:(m + 1) * P, :], in_=zf[:])
                        else:
                            z8 = z8p.tile([P, DH], f8, name=f"z8_{l}_{m}", tag="z8")
                            nc.scalar.activation(out=z8[:], in_=tsum[:], func=RELU)
                            mgi = next(i for i, (a, b) in enumerate(AGS)
                                       if a <= m < b)
                            nc.sync.dma_start(
                                out=zloc8[l][mgi][:, m - AGS[mgi][0], :],
                                in_=z8[:])
                            for gi, (a, b) in enumerate(AGS):
                                if m != b - 1:
                                    continue
                                nch = b - a
                                nc.gpsimd.collective_compute(
                                    "AllGather",
                                    mybir.AluOpType.bypass,
                                    replica_groups=[list(range(NCORES))],
                                    ins=[zloc8[l][gi][:, :, :].opt()],
                                    outs=[zfull8[l][gi][:, :].opt()],
                                )
                                # scatter gathered blocks into zsb8
                                for c in range(NCORES):
                                    g = MT * c + a
                                    nc.scalar.dma_start(
                                        out=zsb8[:, g:g + nch, :],
                                        in_=zfull8[l][gi][c * P:(c + 1) * P, :]
                                            .rearrange("p (g f) -> p g f", g=nch))

    nc.compile()
    return nc


# --------------------------------------------------------------------------
# entry point
# --------------------------------------------------------------------------

def _make_in_maps(inputs, cfg, a8s):
    DIN, DH, L, N = cfg["DIN"], cfg["DH"], cfg["L"], cfg["N"]
    shared = {"x8": _prep_x8(inputs["x"], N, DIN),
              "ccw": np.zeros((P, 64), E4M3)}
    for l in range(L):
        shared[f"w1b_{l}"] = np.asarray(
            inputs[f"w1_{l}"], np.float32).astype(BF16)
        shared[f"w2b_{l}"] = np.asarray(
            inputs[f"w2_{l}"], np.float32).astype(BF16)
        shared[f"b1_{l}"] = np.asarray(
            inputs[f"b1_{l}"], np.float32).reshape(DH, 1)
        b2 = np.asarray(inputs[f"b2_{l}"], np.float32)
        shared[f"b2b_{l}"] = np.ascontiguousarray(
            np.broadcast_to(b2[None, :], (P, DH)))

    in_maps = []
    for c in range(NCORES):
        m = dict(shared)
        m["a8"] = a8s[c]
        in_maps.append(m)
    return in_maps


def get_program(inputs):
    cfg = _config(inputs)
    a8s = _prep_adj(inputs["edge_index"], cfg["N"])
    key = (cfg["DIN"], cfg["DH"], cfg["L"])
    if key not in _BUILD_CACHE:
        _BUILD_CACHE[key] = _build(cfg["DIN"], cfg["DH"], cfg["L"])
    nc = _BUILD_CACHE[key]
    in_maps = _make_in_maps(inputs, cfg, a8s)
    return nc, in_maps, cfg


def kernel(**inputs):
    nc, in_maps, cfg = get_program(inputs)

    if os.environ.get("KERNEL_USE_SIM"):
        from concourse.bass_interp import MultiCoreSim
        sim = MultiCoreSim(nc, num_cores=NCORES)
        cores = list(sim.cores.values())
        for cid, cs in enumerate(cores):
            for name, val in in_maps[cid].items():
                cs.tensor(name)[:] = val
        sim.simulate(check_with_hw=False)
        parts = [np.asarray(cs.tensor("zout")) for cs in cores]
    else:
        from concourse import bass_utils
        res = bass_utils.run_bass_kernel_spmd(
            nc, in_maps, core_ids=list(range(NCORES)),
            trace=bool(os.environ.get("KERNEL_TRACE")),
        )
        kernel.last_results = res
        parts = [res.results[c]["zout"] for c in range(NCORES)]

    N = cfg["N"]
    rows = []
    for c in range(NCORES):
        lo = c * NPC
        take = min(NPC, N - lo)
        if take > 0:
            rows.append(np.asarray(parts[c][:take], dtype=np.float32))
    return np.concatenate(rows, axis=0)
